# revision 11
# baseline (speedup 1.0000x reference)
"""DFSMN layer Trainium2 kernel (8-core SPMD, batch-parallel).

Math: per batch b,
  h = x @ W^T + b_lin                      [L, H]
  out_pre[t] = h[t] + mem[t] + fut[t]  ==  (M @ h)[t]
    with M [L, L] banded: identity + past taps (50) + future taps (5),
    taps are scalars per lag: wm = mem_w.sum(-1), wf = la_w.sum(-1).
  out = LayerNorm_H(out_pre) * gamma + beta

On device (per core = one batch), all bf16 matmuls, fp32 PSUM:
  g tiles  = x @ W^T + b  produced on 64-SHIFTED time boundaries
             G_i = [i*128-64, i*128+64); the two half-empty edge windows are
             merged into one physical tile (partitions 0..63 = t in
             [1984,2048), partitions 64..127 = t in [0,64)).  Bias is folded
             by a DVE broadcast-add during PSUM evacuation (M@(g+1 b^T) =
             M@g + s b^T, which is exactly the reference bias path).
  band     = each aligned output tile j needs source window
             [j*128-50, j*128+133) which fits in G_j u G_{j+1}: only TWO
             128-contract band matmuls per (tile, H-chunk) instead of three.
             Band blocks are slices of the true M, so structural zeros mask
             the "wrong" halves of the merged edge tile.
  out      = (pre - mean) * rsqrt(var + eps) via bn_stats/bn_aggr; the final
             scale/shift runs on GpSimd (Pool) to keep DVE off the critical
             path.
"""
import numpy as np
import ml_dtypes

MEM, LA, EPS = 50, 5, 1e-5
B, L, D, H = 8, 2048, 1024, 2048
NCORES = 8
PT = 128              # time tile (partition dim)
TB = L // PT          # 16 time tiles
DC = D // PT          # 8 contract chunks
HN = 512              # matmul moving free dim
HC = H // HN          # 4 H chunks
SHIFT = 64            # g-tile shift

_cached = {}
last_exec_time_ns = None


def _band_matrix(wm, wf):
    """M [L, L] fp32: out_pre = M @ h."""
    M = np.zeros((L, L), np.float32)
    idx = np.arange(L)
    M[idx, idx] = 1.0
    for t in range(L):
        if t < MEM:
            M[t, :t] += wm[:t]
        else:
            M[t, t - MEM:t] += wm
        hi = min(t + LA, L - 1)
        if hi >= t + 1:
            M[t, t + 1:hi + 1] += wf[:hi - t]
    return M


def _t_of(i, p):
    """Global time index held at free-col/partition p of shifted g tile i."""
    if i == 0:
        return 1984 + p if p < SHIFT else p - SHIFT
    return i * PT - SHIFT + p


def _build_nc(reps=1, loop_k=None):
    from concourse import bacc
    import concourse.mybir as mybir
    import concourse.tile as tile

    dt = mybir.dt.bfloat16
    f32 = mybir.dt.float32

    nc = bacc.Bacc(None, target_bir_lowering=False)
    # x shipped transposed on shifted tile boundaries: [TB, D, PT]; tile i's
    # free col p holds x[t(i,p), :] (see _t_of).
    xtT = nc.declare_dram_parameter("xtT", [TB, D, PT], dt, isOutput=False)
    wT = nc.declare_dram_parameter("wT", [D, H], dt, isOutput=False)
    # band blocks: mtB[ki, j, 0, m] = M[j*128+m, j*128-64+ki]   (L source)
    #              mtB[ki, j, 1, m] = M[j*128+m, j*128+64+ki]   (R source)
    mtB = nc.declare_dram_parameter("mtB", [PT, TB, 2, PT], dt, isOutput=False)
    bvB = nc.declare_dram_parameter("bvB", [PT, H], dt, isOutput=False)
    out = nc.declare_dram_parameter("out", [L, H], f32, isOutput=True)

    with tile.TileContext(nc) as tc:
        with tc.tile_pool(name="const", bufs=1) as const, \
             tc.tile_pool(name="gm", bufs=1) as gmp, \
             tc.tile_pool(name="gpool", bufs=5) as gpool, \
             tc.tile_pool(name="opool", bufs=2) as opool, \
             tc.tile_pool(name="ln", bufs=2) as ln, \
             tc.tile_pool(name="psg", bufs=6, space="PSUM") as psg, \
             tc.tile_pool(name="psp", bufs=2, space="PSUM") as psp:

            # --- input loads: interleave W chunks and x tiles across the two
            # HWDGE queues (sync/scalar) so the first main chain's operands
            # (wt0..7 + xt0) land as early as possible; mt/bvb ride the
            # gpsimd SWDGE queue concurrently.
            wt_tiles = [const.tile([PT, H], dt, tag=f"wt{dc}", name=f"wt{dc}")
                        for dc in range(DC)]
            xt_tiles = [const.tile([PT, DC, PT], dt, tag=f"xt{i}", name=f"xt{i}")
                        for i in range(TB)]
            # xt0 first on scalar, then W alternating both queues, then the
            # remaining x tiles: chain0's deps (wt0..7+xt0) land as early as
            # possible; mt/bvb (needed only by band(0)/g-evac) ride the
            # gpsimd SWDGE queue and are emitted last so they don't delay
            # the critical W/x transfers.
            nc.scalar.dma_start(
                out=xt_tiles[0],
                in_=xtT[0].rearrange("(dc p) t -> p dc t", p=PT))
            for dc in range(DC):
                eng = nc.sync if dc % 2 == 0 else nc.scalar
                eng.dma_start(out=wt_tiles[dc],
                              in_=wT[dc * PT:(dc + 1) * PT, :])
            for i in range(1, TB):
                eng = nc.sync if i % 2 == 0 else nc.scalar
                eng.dma_start(out=xt_tiles[i],
                              in_=xtT[i].rearrange("(dc p) t -> p dc t", p=PT))
            mt_t = const.tile([PT, TB, 2, PT], dt, tag="mt")
            bvb_t = const.tile([PT, H], dt, tag="bvb")
            nc.gpsimd.dma_start(out=bvb_t, in_=bvB[:, :])
            nc.gpsimd.dma_start(out=mt_t, in_=mtB[:, :, :, :])
            eps_t = const.tile([PT, 1], f32, tag="eps")
            nc.vector.memset(eps_t, EPS)

            args = (nc, mybir, xt_tiles, wt_tiles, mt_t, bvb_t, eps_t,
                    gmp, gpool, opool, ln, psg, psp, out)
            if loop_k is not None:
                with tc.For_i(0, loop_k, 1):
                    _emit_body(*args)
            else:
                for _rep in range(reps):
                    _emit_body(*args)
    nc.finalize()
    return nc


def _emit_body(nc, mybir, xt_tiles, wt_tiles, mt_t, bvb_t, eps_t,
               gmp, gpool, opool, ln, psg, psp, out):
    dt = mybir.dt.bfloat16
    f32 = mybir.dt.float32
    sub = mybir.AluOpType.subtract
    mult = mybir.AluOpType.mult
    add = mybir.AluOpType.add

    g_tiles = [None] * TB

    def emit_band(j):
        gL = g_tiles[j]
        gR = g_tiles[(j + 1) % TB]
        stats = ln.tile([PT, HC, 6], f32, tag="stats")
        presb = []
        for hc in range(HC):
            pre = psp.tile([PT, HN], f32, tag="pre")
            nc.tensor.matmul(pre, mt_t[:, j, 0, :], gL[hc],
                             start=True, stop=False)
            nc.tensor.matmul(pre, mt_t[:, j, 1, :], gR[hc],
                             start=False, stop=True)
            # ScalarE evacuates PSUM (it sits closest to PSUM); bn_stats
            # then reads SBUF on DVE.
            psb = opool.tile([PT, HN], f32, tag=f"psb{hc}")
            nc.scalar.copy(out=psb, in_=pre)
            nc.vector.bn_stats(out=stats[:, hc, :], in_=psb)
            presb.append(psb)
        mv = ln.tile([PT, 2], f32, tag="mv")
        nc.vector.bn_aggr(out=mv, in_=stats)
        rstd = ln.tile([PT, 1], f32, tag="rstd")
        nc.scalar.activation(
            out=rstd, in_=mv[:, 1:2],
            func=mybir.ActivationFunctionType.Sqrt,
            bias=eps_t, scale=1.0)
        nc.vector.reciprocal(out=rstd, in_=rstd)
        for hc in range(HC):
            # (pre - mean) * rstd on GpSimd: keeps DVE/ScalarE free.  Output
            # DMA per H-chunk so the last tile's store overlaps its LN.
            o = opool.tile([PT, HN], f32, tag=f"o{hc}", name=f"o{hc}")
            nc.gpsimd.tensor_scalar(
                out=o, in0=presb[hc],
                scalar1=mv[:, 0:1], scalar2=rstd,
                op0=sub, op1=mult)
            eng = nc.sync if (j * HC + hc) % 2 == 0 else nc.scalar
            eng.dma_start(
                out=out[j * PT:(j + 1) * PT, hc * HN:(hc + 1) * HN], in_=o)

    for i in range(TB):
        # main chain i -> shifted g tile i.  dc-outer order: the stationary
        # x chunk is reused across the 4 H-chunk PSUM chains (fewer
        # LDWEIGHTS), chains run in 4 PSUM banks concurrently.
        pgs = [psg.tile([PT, HN], f32, tag="pg", name=f"pg{i}_{hc}")
               for hc in range(HC)]
        for dc in range(DC):
            for hc in range(HC):
                nc.tensor.matmul(
                    pgs[hc],
                    xt_tiles[i][:, dc, :],
                    wt_tiles[dc][:, hc * HN:(hc + 1) * HN],
                    start=(dc == 0), stop=(dc == DC - 1))
        gch = []
        for hc in range(HC):
            pool = gmp if i == 0 else gpool
            g = pool.tile([PT, HN], dt, tag=(f"gm{hc}" if i == 0 else f"g{hc}"))
            # evacuate PSUM on DVE with the bias broadcast-add folded in
            nc.vector.tensor_tensor(
                out=g, in0=pgs[hc], in1=bvb_t[:, hc * HN:(hc + 1) * HN],
                op=add)
            gch.append(g)
        g_tiles[i] = gch
        # band(j) waits one extra chain (emitted after chain j+2) so the DVE
        # evacuation of g(j+1) has a full chain of PE work to hide under.
        if i >= 2:
            emit_band(i - 2)
    emit_band(TB - 2)
    emit_band(TB - 1)


def _get_runner(reps=1):
    """Compile once; return (run_fn, in_names, out_names, zero_outs, mesh)."""
    key = ("runner", reps)
    if key in _cached:
        return _cached[key]

    import jax
    from jax.experimental.shard_map import shard_map
    from jax.sharding import Mesh, PartitionSpec
    import concourse.mybir as mybir
    from concourse import bass2jax

    if isinstance(reps, tuple):  # ("loop", K): hardware For_i timing variant
        nc = _build_nc(loop_k=reps[1])
    else:
        nc = _build_nc(reps)
    bass2jax.install_neuronx_cc_hook()

    partition_name = nc.partition_id_tensor.name if nc.partition_id_tensor else None
    in_names, out_names, out_avals, zero_outs = [], [], [], []
    for alloc in nc.m.functions[0].allocations:
        if not isinstance(alloc, mybir.MemoryLocationSet):
            continue
        name = alloc.memorylocations[0].name
        if alloc.kind == "ExternalInput":
            if name != partition_name:
                in_names.append(name)
        elif alloc.kind == "ExternalOutput":
            out_names.append(name)
            shape = tuple(alloc.tensor_shape)
            dtype = mybir.dt.np(alloc.dtype)
            out_avals.append(jax.core.ShapedArray(shape, dtype))
            zero_outs.append(np.zeros(shape, dtype))
    n_params = len(in_names)
    all_names = in_names + out_names
    if partition_name is not None:
        all_names.append(partition_name)

    def _body(*args):
        operands = list(args)
        if partition_name is not None:
            operands.append(bass2jax.partition_id_tensor())
        outs = bass2jax._bass_exec_p.bind(
            *operands,
            out_avals=tuple(out_avals),
            in_names=tuple(all_names),
            out_names=tuple(out_names),
            lowering_input_output_aliases=(),
            sim_require_finite=True,
            sim_require_nnan=True,
            nc=nc,
        )
        return tuple(outs)

    devices = jax.devices()[:NCORES]
    assert len(devices) == NCORES, f"need {NCORES} devices, have {len(jax.devices())}"
    mesh = Mesh(np.asarray(devices), ("core",))
    n_outs = len(out_names)
    fn = jax.jit(shard_map(
        _body, mesh=mesh,
        in_specs=(PartitionSpec("core"),) * (n_params + n_outs),
        out_specs=(PartitionSpec("core"),) * n_outs,
        check_rep=False))

    _cached[key] = (fn, in_names, out_names, zero_outs, mesh)
    return _cached[key]


def _prepare_in_arrays(x, W_lin, b_lin, wm, wf):
    """Host prep: per-core inputs concatenated over the core axis (axis 0)."""
    bf16 = ml_dtypes.bfloat16
    M = _band_matrix(wm, wf)

    # band blocks from the true M (out-of-range source columns are zero)
    mt_host = np.zeros((PT, TB, 2, PT), np.float32)
    for j in range(TB):
        rows = slice(j * PT, (j + 1) * PT)
        for side, base in ((0, j * PT - SHIFT), (1, j * PT + SHIFT)):
            lo = max(0, base)
            hi = min(L, base + PT)
            if hi > lo:
                # mt[ki, j, side, m] = M[j*128+m, base+ki]
                mt_host[lo - base:hi - base, j, side, :] = M[rows, lo:hi].T

    per_core = {
        "wT": np.ascontiguousarray(W_lin.T).astype(bf16),
        "mtB": mt_host.astype(bf16),
        "bvB": np.ascontiguousarray(
            np.broadcast_to(b_lin.reshape(1, H), (PT, H))).astype(bf16),
    }

    # shifted x tiles: xt[i][d, p] = x[b, t(i,p), d]
    tmap = np.empty((TB, PT), np.int64)
    for i in range(TB):
        for p in range(PT):
            tmap[i, p] = _t_of(i, p)
    arrays = {}
    xb = x.reshape(B, L, D)[:, tmap, :]                  # [B, TB, PT, D]
    xt = np.ascontiguousarray(xb.transpose(0, 1, 3, 2)).astype(bf16)
    arrays["xtT"] = xt.reshape(B * TB, D, PT)
    for name, arr in per_core.items():
        arrays[name] = np.concatenate([arr] * NCORES, axis=0)
    return arrays


def _run(arrays):
    fn, in_names, out_names, zero_outs, _ = _get_runner()
    global_zero = [np.concatenate([z] * NCORES, axis=0) for z in zero_outs]
    args = [arrays[n] for n in in_names] + global_zero
    outs = fn(*args)
    return {n: np.asarray(o) for n, o in zip(out_names, outs)}


def kernel(x, W_lin, b_lin, mem_w, la_w, gamma, beta):
    x = np.asarray(x, np.float32)
    W_lin = np.asarray(W_lin, np.float32)
    b_lin = np.asarray(b_lin, np.float32)
    wm = np.asarray(mem_w, np.float32).sum(axis=-1, dtype=np.float32)
    wf = np.asarray(la_w, np.float32).sum(axis=-1, dtype=np.float32)
    gamma = np.asarray(gamma, np.float32)
    beta = np.asarray(beta, np.float32)

    arrays = _prepare_in_arrays(x, W_lin, b_lin, wm, wf)
    outs = _run(arrays)
    out = outs["out"].reshape(NCORES, L, H)

    # gamma/beta affine (trivial for the spec's ones/zeros fills; exact in general)
    if not np.all(gamma == 1.0):
        out = out * gamma[None, None, :]
    if not np.all(beta == 0.0):
        out = out + beta[None, None, :]
    return np.ascontiguousarray(out.astype(np.float32))


# revision 15
# speedup vs baseline: 2.8548x; 2.8548x over previous
"""DFSMN layer Trainium2 kernel (8-core SPMD, batch-parallel).

Math: per batch b,
  h = x @ W^T + b_lin                      [L, H]
  out_pre[t] = h[t] + mem[t] + fut[t]  ==  (M @ h)[t]
    with M [L, L] banded: identity + past taps (50) + future taps (5),
    taps are scalars per lag: wm = mem_w.sum(-1), wf = la_w.sum(-1).
  out = LayerNorm_H(out_pre) * gamma + beta

On device (per core = one batch), all bf16 matmuls, fp32 PSUM:
  g tiles  = x @ W^T + b  produced on 64-SHIFTED time boundaries
             G_i = [i*128-64, i*128+64); the two half-empty edge windows are
             merged into one physical tile (partitions 0..63 = t in
             [1984,2048), partitions 64..127 = t in [0,64)).  Bias is folded
             by a DVE broadcast-add during PSUM evacuation (M@(g+1 b^T) =
             M@g + s b^T, which is exactly the reference bias path).
  band     = each aligned output tile j needs source window
             [j*128-50, j*128+133) which fits in G_j u G_{j+1}: only TWO
             128-contract band matmuls per (tile, H-chunk) instead of three.
             Band blocks are slices of the true M, so structural zeros mask
             the "wrong" halves of the merged edge tile.
  out      = (pre - mean) * rsqrt(var + eps) via bn_stats/bn_aggr; the final
             scale/shift runs on GpSimd (Pool) to keep DVE off the critical
             path.
"""
import numpy as np
import ml_dtypes

MEM, LA, EPS = 50, 5, 1e-5
B, L, D, H = 8, 2048, 1024, 2048
NCORES = 8
PT = 128              # time tile (partition dim)
TB = L // PT          # 16 time tiles
DC = D // PT          # 8 contract chunks
HN = 512              # matmul moving free dim
HC = H // HN          # 4 H chunks
SHIFT = 64            # g-tile shift

_cached = {}
last_exec_time_ns = None


def _band_matrix(wm, wf):
    """M [L, L] fp32: out_pre = M @ h."""
    M = np.zeros((L, L), np.float32)
    idx = np.arange(L)
    M[idx, idx] = 1.0
    for t in range(L):
        if t < MEM:
            M[t, :t] += wm[:t]
        else:
            M[t, t - MEM:t] += wm
        hi = min(t + LA, L - 1)
        if hi >= t + 1:
            M[t, t + 1:hi + 1] += wf[:hi - t]
    return M


def _t_of(i, p):
    """Global time index held at free-col/partition p of shifted g tile i."""
    if i == 0:
        return 1984 + p if p < SHIFT else p - SHIFT
    return i * PT - SHIFT + p


def _build_nc(reps=1, loop_k=None):
    from concourse import bacc
    import concourse.mybir as mybir
    import concourse.tile as tile

    dt = mybir.dt.bfloat16
    f32 = mybir.dt.float32

    nc = bacc.Bacc(None, target_bir_lowering=False)
    # x shipped transposed on shifted tile boundaries: [TB, D, PT]; tile i's
    # free col p holds x[t(i,p), :] (see _t_of).
    xtT = nc.declare_dram_parameter("xtT", [TB, D, PT], dt, isOutput=False)
    wT = nc.declare_dram_parameter("wT", [D, H], dt, isOutput=False)
    # band blocks: mtB[ki, j, 0, m] = M[j*128+m, j*128-64+ki]   (L source)
    #              mtB[ki, j, 1, m] = M[j*128+m, j*128+64+ki]   (R source)
    mtB = nc.declare_dram_parameter("mtB", [PT, TB, 2, PT], dt, isOutput=False)
    bvB = nc.declare_dram_parameter("bvB", [PT, H], dt, isOutput=False)
    out = nc.declare_dram_parameter("out", [L, H], f32, isOutput=True)

    with tile.TileContext(nc) as tc:
        with tc.tile_pool(name="const", bufs=1) as const, \
             tc.tile_pool(name="gm", bufs=1) as gmp, \
             tc.tile_pool(name="gpool", bufs=5) as gpool, \
             tc.tile_pool(name="opool", bufs=2) as opool, \
             tc.tile_pool(name="ln", bufs=2) as ln, \
             tc.tile_pool(name="psg", bufs=4, space="PSUM") as psg, \
             tc.tile_pool(name="psp", bufs=4, space="PSUM") as psp:

            # --- input loads: interleave W chunks and x tiles across the two
            # HWDGE queues (sync/scalar) so the first main chain's operands
            # (wt0..7 + xt0) land as early as possible; mt/bvb ride the
            # gpsimd SWDGE queue concurrently.
            wt_tiles = [const.tile([PT, H], dt, tag=f"wt{dc}", name=f"wt{dc}")
                        for dc in range(DC)]
            xt_tiles = [const.tile([PT, DC, PT], dt, tag=f"xt{i}", name=f"xt{i}")
                        for i in range(TB)]
            # xt0 first on scalar, then W alternating both queues, then the
            # remaining x tiles: chain0's deps (wt0..7+xt0) land as early as
            # possible; mt/bvb (needed only by band(0)/g-evac) ride the
            # gpsimd SWDGE queue and are emitted last so they don't delay
            # the critical W/x transfers.
            nc.scalar.dma_start(
                out=xt_tiles[0],
                in_=xtT[0].rearrange("(dc p) t -> p dc t", p=PT))
            for dc in range(DC):
                eng = nc.sync if dc % 2 == 0 else nc.scalar
                eng.dma_start(out=wt_tiles[dc],
                              in_=wT[dc * PT:(dc + 1) * PT, :])
            for i in range(1, TB):
                eng = nc.sync if i % 2 == 0 else nc.scalar
                eng.dma_start(out=xt_tiles[i],
                              in_=xtT[i].rearrange("(dc p) t -> p dc t", p=PT))
            mt_t = const.tile([PT, TB, 2, PT], dt, tag="mt")
            bvb_t = const.tile([PT, H], dt, tag="bvb")
            nc.gpsimd.dma_start(out=bvb_t, in_=bvB[:, :])
            nc.gpsimd.dma_start(out=mt_t, in_=mtB[:, :, :, :])
            eps_t = const.tile([PT, 1], f32, tag="eps")
            nc.vector.memset(eps_t, EPS)

            args = (nc, mybir, xt_tiles, wt_tiles, mt_t, bvb_t, eps_t,
                    gmp, gpool, opool, ln, psg, psp, out)
            if loop_k is not None:
                with tc.For_i(0, loop_k, 1):
                    _emit_body(*args)
            else:
                for _rep in range(reps):
                    _emit_body(*args)
    nc.finalize()
    return nc


def _emit_body(nc, mybir, xt_tiles, wt_tiles, mt_t, bvb_t, eps_t,
               gmp, gpool, opool, ln, psg, psp, out):
    dt = mybir.dt.bfloat16
    f32 = mybir.dt.float32
    sub = mybir.AluOpType.subtract
    mult = mybir.AluOpType.mult
    add = mybir.AluOpType.add

    g_tiles = [None] * TB

    def emit_band(j):
        gL = g_tiles[j]
        gR = g_tiles[(j + 1) % TB]
        stats = ln.tile([PT, HC, 6], f32, tag="stats")
        pres = []
        for hc in range(HC):
            pre = psp.tile([PT, HN], f32, tag="pre")
            nc.tensor.matmul(pre, mt_t[:, j, 0, :], gL[hc],
                             start=True, stop=False)
            nc.tensor.matmul(pre, mt_t[:, j, 1, :], gR[hc],
                             start=False, stop=True)
            # LN runs straight out of PSUM: bn_stats (DVE) now, the final
            # scale/shift (ScalarE) later — no evacuation copy at all.
            nc.vector.bn_stats(out=stats[:, hc, :], in_=pre)
            pres.append(pre)
        mv = ln.tile([PT, 2], f32, tag="mv")
        nc.vector.bn_aggr(out=mv, in_=stats)
        rstd = ln.tile([PT, 1], f32, tag="rstd")
        nc.scalar.activation(
            out=rstd, in_=mv[:, 1:2],
            func=mybir.ActivationFunctionType.Sqrt,
            bias=eps_t, scale=1.0)
        nc.vector.reciprocal(out=rstd, in_=rstd)
        # nmr = -mean * rstd so the final scale/shift can run on ScalarE as
        # one Identity activation: o = pre * rstd + nmr.
        nmr = ln.tile([PT, 1], f32, tag="nmr")
        nc.vector.tensor_scalar(
            out=nmr, in0=mv[:, 0:1], scalar1=rstd, scalar2=-1.0,
            op0=mult, op1=mult)
        for hc in range(HC):
            # Output DMA per H-chunk so the last tile's store overlaps its LN.
            o = opool.tile([PT, HN], f32, tag=f"o{hc}", name=f"o{hc}")
            nc.scalar.activation(
                out=o, in_=pres[hc],
                func=mybir.ActivationFunctionType.Identity,
                bias=nmr, scale=rstd)
            eng = nc.sync if (j * HC + hc) % 2 == 0 else nc.scalar
            eng.dma_start(
                out=out[j * PT:(j + 1) * PT, hc * HN:(hc + 1) * HN], in_=o)

    for i in range(TB):
        # main chain i -> shifted g tile i.  dc-outer order: the stationary
        # x chunk is reused across the 4 H-chunk PSUM chains (fewer
        # LDWEIGHTS), chains run in 4 PSUM banks concurrently.
        pgs = [psg.tile([PT, HN], f32, tag="pg", name=f"pg{i}_{hc}")
               for hc in range(HC)]
        for dc in range(DC):
            for hc in range(HC):
                nc.tensor.matmul(
                    pgs[hc],
                    xt_tiles[i][:, dc, :],
                    wt_tiles[dc][:, hc * HN:(hc + 1) * HN],
                    start=(dc == 0), stop=(dc == DC - 1))
        gch = []
        for hc in range(HC):
            pool = gmp if i == 0 else gpool
            g = pool.tile([PT, HN], dt, tag=(f"gm{hc}" if i == 0 else f"g{hc}"))
            # evacuate PSUM on DVE with the bias broadcast-add folded in
            nc.vector.tensor_tensor(
                out=g, in0=pgs[hc], in1=bvb_t[:, hc * HN:(hc + 1) * HN],
                op=add)
            gch.append(g)
        g_tiles[i] = gch
        # band(j) waits one extra chain (emitted after chain j+2) so the DVE
        # evacuation of g(j+1) has a full chain of PE work to hide under.
        if i >= 2:
            emit_band(i - 2)
    emit_band(TB - 2)
    emit_band(TB - 1)


def _get_runner(reps=1):
    """Compile once; return (run_fn, in_names, out_names, zero_outs, mesh)."""
    key = ("runner", reps)
    if key in _cached:
        return _cached[key]

    import jax
    from jax.experimental.shard_map import shard_map
    from jax.sharding import Mesh, PartitionSpec
    import concourse.mybir as mybir
    from concourse import bass2jax

    if isinstance(reps, tuple):  # ("loop", K): hardware For_i timing variant
        nc = _build_nc(loop_k=reps[1])
    else:
        nc = _build_nc(reps)
    bass2jax.install_neuronx_cc_hook()

    partition_name = nc.partition_id_tensor.name if nc.partition_id_tensor else None
    in_names, out_names, out_avals, zero_outs = [], [], [], []
    for alloc in nc.m.functions[0].allocations:
        if not isinstance(alloc, mybir.MemoryLocationSet):
            continue
        name = alloc.memorylocations[0].name
        if alloc.kind == "ExternalInput":
            if name != partition_name:
                in_names.append(name)
        elif alloc.kind == "ExternalOutput":
            out_names.append(name)
            shape = tuple(alloc.tensor_shape)
            dtype = mybir.dt.np(alloc.dtype)
            out_avals.append(jax.core.ShapedArray(shape, dtype))
            zero_outs.append(np.zeros(shape, dtype))
    n_params = len(in_names)
    all_names = in_names + out_names
    if partition_name is not None:
        all_names.append(partition_name)

    def _body(*args):
        operands = list(args)
        if partition_name is not None:
            operands.append(bass2jax.partition_id_tensor())
        outs = bass2jax._bass_exec_p.bind(
            *operands,
            out_avals=tuple(out_avals),
            in_names=tuple(all_names),
            out_names=tuple(out_names),
            lowering_input_output_aliases=(),
            sim_require_finite=True,
            sim_require_nnan=True,
            nc=nc,
        )
        return tuple(outs)

    devices = jax.devices()[:NCORES]
    assert len(devices) == NCORES, f"need {NCORES} devices, have {len(jax.devices())}"
    mesh = Mesh(np.asarray(devices), ("core",))
    n_outs = len(out_names)
    fn = jax.jit(shard_map(
        _body, mesh=mesh,
        in_specs=(PartitionSpec("core"),) * (n_params + n_outs),
        out_specs=(PartitionSpec("core"),) * n_outs,
        check_rep=False))

    _cached[key] = (fn, in_names, out_names, zero_outs, mesh)
    return _cached[key]


def _prepare_in_arrays(x, W_lin, b_lin, wm, wf):
    """Host prep: per-core inputs concatenated over the core axis (axis 0)."""
    bf16 = ml_dtypes.bfloat16
    M = _band_matrix(wm, wf)

    # band blocks from the true M (out-of-range source columns are zero)
    mt_host = np.zeros((PT, TB, 2, PT), np.float32)
    for j in range(TB):
        rows = slice(j * PT, (j + 1) * PT)
        for side, base in ((0, j * PT - SHIFT), (1, j * PT + SHIFT)):
            lo = max(0, base)
            hi = min(L, base + PT)
            if hi > lo:
                # mt[ki, j, side, m] = M[j*128+m, base+ki]
                mt_host[lo - base:hi - base, j, side, :] = M[rows, lo:hi].T

    per_core = {
        "wT": np.ascontiguousarray(W_lin.T).astype(bf16),
        "mtB": mt_host.astype(bf16),
        "bvB": np.ascontiguousarray(
            np.broadcast_to(b_lin.reshape(1, H), (PT, H))).astype(bf16),
    }

    # shifted x tiles: xt[i][d, p] = x[b, t(i,p), d]
    tmap = np.empty((TB, PT), np.int64)
    for i in range(TB):
        for p in range(PT):
            tmap[i, p] = _t_of(i, p)
    arrays = {}
    xb = x.reshape(B, L, D)[:, tmap, :]                  # [B, TB, PT, D]
    xt = np.ascontiguousarray(xb.transpose(0, 1, 3, 2)).astype(bf16)
    arrays["xtT"] = xt.reshape(B * TB, D, PT)
    for name, arr in per_core.items():
        arrays[name] = np.concatenate([arr] * NCORES, axis=0)
    return arrays


def _run(arrays):
    fn, in_names, out_names, zero_outs, _ = _get_runner()
    global_zero = [np.concatenate([z] * NCORES, axis=0) for z in zero_outs]
    args = [arrays[n] for n in in_names] + global_zero
    outs = fn(*args)
    return {n: np.asarray(o) for n, o in zip(out_names, outs)}


def kernel(x, W_lin, b_lin, mem_w, la_w, gamma, beta):
    x = np.asarray(x, np.float32)
    W_lin = np.asarray(W_lin, np.float32)
    b_lin = np.asarray(b_lin, np.float32)
    wm = np.asarray(mem_w, np.float32).sum(axis=-1, dtype=np.float32)
    wf = np.asarray(la_w, np.float32).sum(axis=-1, dtype=np.float32)
    gamma = np.asarray(gamma, np.float32)
    beta = np.asarray(beta, np.float32)

    arrays = _prepare_in_arrays(x, W_lin, b_lin, wm, wf)
    outs = _run(arrays)
    out = outs["out"].reshape(NCORES, L, H)

    # gamma/beta affine (trivial for the spec's ones/zeros fills; exact in general)
    if not np.all(gamma == 1.0):
        out = out * gamma[None, None, :]
    if not np.all(beta == 0.0):
        out = out + beta[None, None, :]
    return np.ascontiguousarray(out.astype(np.float32))


# revision 16
# speedup vs baseline: 3.1570x; 1.1058x over previous
"""DFSMN layer Trainium2 kernel (8-core SPMD, batch-parallel).

Math: per batch b,
  h = x @ W^T + b_lin                      [L, H]
  out_pre[t] = h[t] + mem[t] + fut[t]  ==  (M @ h)[t]
    with M [L, L] banded: identity + past taps (50) + future taps (5),
    taps are scalars per lag: wm = mem_w.sum(-1), wf = la_w.sum(-1).
  out = LayerNorm_H(out_pre) * gamma + beta

On device (per core = one batch), all bf16 matmuls, fp32 PSUM:
  g tiles  = x @ W^T + b  produced on 64-SHIFTED time boundaries
             G_i = [i*128-64, i*128+64); the two half-empty edge windows are
             merged into one physical tile (partitions 0..63 = t in
             [1984,2048), partitions 64..127 = t in [0,64)).  Bias is folded
             by a DVE broadcast-add during PSUM evacuation (M@(g+1 b^T) =
             M@g + s b^T, which is exactly the reference bias path).
  band     = each aligned output tile j needs source window
             [j*128-50, j*128+133) which fits in G_j u G_{j+1}: only TWO
             128-contract band matmuls per (tile, H-chunk) instead of three.
             Band blocks are slices of the true M, so structural zeros mask
             the "wrong" halves of the merged edge tile.
  out      = (pre - mean) * rsqrt(var + eps) via bn_stats/bn_aggr; the final
             scale/shift runs on GpSimd (Pool) to keep DVE off the critical
             path.
"""
import numpy as np
import ml_dtypes

MEM, LA, EPS = 50, 5, 1e-5
B, L, D, H = 8, 2048, 1024, 2048
NCORES = 8
PT = 128              # time tile (partition dim)
TB = L // PT          # 16 time tiles
DC = D // PT          # 8 contract chunks
HN = 512              # matmul moving free dim
HC = H // HN          # 4 H chunks
SHIFT = 64            # g-tile shift

_cached = {}
last_exec_time_ns = None


def _band_matrix(wm, wf):
    """M [L, L] fp32: out_pre = M @ h."""
    M = np.zeros((L, L), np.float32)
    idx = np.arange(L)
    M[idx, idx] = 1.0
    for t in range(L):
        if t < MEM:
            M[t, :t] += wm[:t]
        else:
            M[t, t - MEM:t] += wm
        hi = min(t + LA, L - 1)
        if hi >= t + 1:
            M[t, t + 1:hi + 1] += wf[:hi - t]
    return M


def _t_of(i, p):
    """Global time index held at free-col/partition p of shifted g tile i."""
    if i == 0:
        return 1984 + p if p < SHIFT else p - SHIFT
    return i * PT - SHIFT + p


def _build_nc(reps=1, loop_k=None):
    from concourse import bacc
    import concourse.mybir as mybir
    import concourse.tile as tile

    dt = mybir.dt.bfloat16
    f32 = mybir.dt.float32

    nc = bacc.Bacc(None, target_bir_lowering=False)
    # x shipped transposed on shifted tile boundaries: [TB, D, PT]; tile i's
    # free col p holds x[t(i,p), :] (see _t_of).
    xtT = nc.declare_dram_parameter("xtT", [TB, D, PT], dt, isOutput=False)
    wT = nc.declare_dram_parameter("wT", [D, H], dt, isOutput=False)
    # band blocks: mtB[ki, j, 0, m] = M[j*128+m, j*128-64+ki]   (L source)
    #              mtB[ki, j, 1, m] = M[j*128+m, j*128+64+ki]   (R source)
    mtB = nc.declare_dram_parameter("mtB", [PT, TB, 2, PT], dt, isOutput=False)
    bvB = nc.declare_dram_parameter("bvB", [PT, H], dt, isOutput=False)
    out = nc.declare_dram_parameter("out", [L, H], f32, isOutput=True)

    with tile.TileContext(nc) as tc:
        with tc.tile_pool(name="const", bufs=1) as const, \
             tc.tile_pool(name="gm", bufs=1) as gmp, \
             tc.tile_pool(name="gpool", bufs=5) as gpool, \
             tc.tile_pool(name="opool", bufs=2) as opool, \
             tc.tile_pool(name="ln", bufs=2) as ln, \
             tc.tile_pool(name="psg", bufs=4, space="PSUM") as psg, \
             tc.tile_pool(name="psp", bufs=4, space="PSUM") as psp:

            # --- input loads: interleave W chunks and x tiles across the two
            # HWDGE queues (sync/scalar) so the first main chain's operands
            # (wt0..7 + xt0) land as early as possible; mt/bvb ride the
            # gpsimd SWDGE queue concurrently.
            wt_tiles = [const.tile([PT, H], dt, tag=f"wt{dc}", name=f"wt{dc}")
                        for dc in range(DC)]
            xt_tiles = [const.tile([PT, DC, PT], dt, tag=f"xt{i}", name=f"xt{i}")
                        for i in range(TB)]
            # xt0 first on scalar, then W alternating both queues, then the
            # remaining x tiles: chain0's deps (wt0..7+xt0) land as early as
            # possible; mt/bvb (needed only by band(0)/g-evac) ride the
            # gpsimd SWDGE queue and are emitted last so they don't delay
            # the critical W/x transfers.
            nc.scalar.dma_start(
                out=xt_tiles[0],
                in_=xtT[0].rearrange("(dc p) t -> p dc t", p=PT))
            for dc in range(DC):
                eng = nc.sync if dc % 2 == 0 else nc.scalar
                eng.dma_start(out=wt_tiles[dc],
                              in_=wT[dc * PT:(dc + 1) * PT, :])
            for i in range(1, TB):
                eng = nc.sync if i % 2 == 0 else nc.scalar
                eng.dma_start(out=xt_tiles[i],
                              in_=xtT[i].rearrange("(dc p) t -> p dc t", p=PT))
            mt_t = const.tile([PT, TB, 2, PT], dt, tag="mt")
            bvb_t = const.tile([PT, H], dt, tag="bvb")
            nc.gpsimd.dma_start(out=bvb_t, in_=bvB[:, :])
            nc.gpsimd.dma_start(out=mt_t, in_=mtB[:, :, :, :])
            eps_t = const.tile([PT, 1], f32, tag="eps")
            nc.vector.memset(eps_t, EPS)

            args = (nc, mybir, xt_tiles, wt_tiles, mt_t, bvb_t, eps_t,
                    gmp, gpool, opool, ln, psg, psp, out)
            if loop_k is not None:
                with tc.For_i(0, loop_k, 1):
                    _emit_body(*args)
            else:
                for _rep in range(reps):
                    _emit_body(*args)
    nc.finalize()
    return nc


def _emit_body(nc, mybir, xt_tiles, wt_tiles, mt_t, bvb_t, eps_t,
               gmp, gpool, opool, ln, psg, psp, out):
    dt = mybir.dt.bfloat16
    f32 = mybir.dt.float32
    sub = mybir.AluOpType.subtract
    mult = mybir.AluOpType.mult
    add = mybir.AluOpType.add

    g_tiles = [None] * TB

    def emit_band(j):
        gL = g_tiles[j]
        gR = g_tiles[(j + 1) % TB]
        stats = ln.tile([PT, HC, 6], f32, tag="stats")
        pres = [psp.tile([PT, HN], f32, tag="pre", name=f"pre{j}_{hc}")
                for hc in range(HC)]
        # All L matmuls then all R matmuls: the stationary band block is
        # loaded once per group of four instead of per matmul.
        for hc in range(HC):
            nc.tensor.matmul(pres[hc], mt_t[:, j, 0, :], gL[hc],
                             start=True, stop=False)
        for hc in range(HC):
            nc.tensor.matmul(pres[hc], mt_t[:, j, 1, :], gR[hc],
                             start=False, stop=True)
        for hc in range(HC):
            # LN runs straight out of PSUM: bn_stats (DVE) now, the final
            # scale/shift (ScalarE) later — no evacuation copy at all.
            nc.vector.bn_stats(out=stats[:, hc, :], in_=pres[hc])
        mv = ln.tile([PT, 2], f32, tag="mv")
        nc.vector.bn_aggr(out=mv, in_=stats)
        rstd = ln.tile([PT, 1], f32, tag="rstd")
        nc.scalar.activation(
            out=rstd, in_=mv[:, 1:2],
            func=mybir.ActivationFunctionType.Sqrt,
            bias=eps_t, scale=1.0)
        nc.vector.reciprocal(out=rstd, in_=rstd)
        # nmr = -mean * rstd so the final scale/shift can run on ScalarE as
        # one Identity activation: o = pre * rstd + nmr.
        nmr = ln.tile([PT, 1], f32, tag="nmr")
        nc.vector.tensor_scalar(
            out=nmr, in0=mv[:, 0:1], scalar1=rstd, scalar2=-1.0,
            op0=mult, op1=mult)
        for hc in range(HC):
            # Output DMA per H-chunk so the last tile's store overlaps its LN.
            o = opool.tile([PT, HN], f32, tag=f"o{hc}", name=f"o{hc}")
            nc.scalar.activation(
                out=o, in_=pres[hc],
                func=mybir.ActivationFunctionType.Identity,
                bias=nmr, scale=rstd)
            eng = nc.sync if (j * HC + hc) % 2 == 0 else nc.scalar
            eng.dma_start(
                out=out[j * PT:(j + 1) * PT, hc * HN:(hc + 1) * HN], in_=o)

    for i in range(TB):
        # main chain i -> shifted g tile i.  dc-outer order: the stationary
        # x chunk is reused across the 4 H-chunk PSUM chains (fewer
        # LDWEIGHTS), chains run in 4 PSUM banks concurrently.
        pgs = [psg.tile([PT, HN], f32, tag="pg", name=f"pg{i}_{hc}")
               for hc in range(HC)]
        for dc in range(DC):
            for hc in range(HC):
                nc.tensor.matmul(
                    pgs[hc],
                    xt_tiles[i][:, dc, :],
                    wt_tiles[dc][:, hc * HN:(hc + 1) * HN],
                    start=(dc == 0), stop=(dc == DC - 1))
        gch = []
        for hc in range(HC):
            pool = gmp if i == 0 else gpool
            g = pool.tile([PT, HN], dt, tag=(f"gm{hc}" if i == 0 else f"g{hc}"))
            # evacuate PSUM on DVE with the bias broadcast-add folded in
            nc.vector.tensor_tensor(
                out=g, in0=pgs[hc], in1=bvb_t[:, hc * HN:(hc + 1) * HN],
                op=add)
            gch.append(g)
        g_tiles[i] = gch
        # band(j) waits one extra chain (emitted after chain j+2) so the DVE
        # evacuation of g(j+1) has a full chain of PE work to hide under.
        if i >= 2:
            emit_band(i - 2)
    emit_band(TB - 2)
    emit_band(TB - 1)


def _get_runner(reps=1):
    """Compile once; return (run_fn, in_names, out_names, zero_outs, mesh)."""
    key = ("runner", reps)
    if key in _cached:
        return _cached[key]

    import jax
    from jax.experimental.shard_map import shard_map
    from jax.sharding import Mesh, PartitionSpec
    import concourse.mybir as mybir
    from concourse import bass2jax

    if isinstance(reps, tuple):  # ("loop", K): hardware For_i timing variant
        nc = _build_nc(loop_k=reps[1])
    else:
        nc = _build_nc(reps)
    bass2jax.install_neuronx_cc_hook()

    partition_name = nc.partition_id_tensor.name if nc.partition_id_tensor else None
    in_names, out_names, out_avals, zero_outs = [], [], [], []
    for alloc in nc.m.functions[0].allocations:
        if not isinstance(alloc, mybir.MemoryLocationSet):
            continue
        name = alloc.memorylocations[0].name
        if alloc.kind == "ExternalInput":
            if name != partition_name:
                in_names.append(name)
        elif alloc.kind == "ExternalOutput":
            out_names.append(name)
            shape = tuple(alloc.tensor_shape)
            dtype = mybir.dt.np(alloc.dtype)
            out_avals.append(jax.core.ShapedArray(shape, dtype))
            zero_outs.append(np.zeros(shape, dtype))
    n_params = len(in_names)
    all_names = in_names + out_names
    if partition_name is not None:
        all_names.append(partition_name)

    def _body(*args):
        operands = list(args)
        if partition_name is not None:
            operands.append(bass2jax.partition_id_tensor())
        outs = bass2jax._bass_exec_p.bind(
            *operands,
            out_avals=tuple(out_avals),
            in_names=tuple(all_names),
            out_names=tuple(out_names),
            lowering_input_output_aliases=(),
            sim_require_finite=True,
            sim_require_nnan=True,
            nc=nc,
        )
        return tuple(outs)

    devices = jax.devices()[:NCORES]
    assert len(devices) == NCORES, f"need {NCORES} devices, have {len(jax.devices())}"
    mesh = Mesh(np.asarray(devices), ("core",))
    n_outs = len(out_names)
    fn = jax.jit(shard_map(
        _body, mesh=mesh,
        in_specs=(PartitionSpec("core"),) * (n_params + n_outs),
        out_specs=(PartitionSpec("core"),) * n_outs,
        check_rep=False))

    _cached[key] = (fn, in_names, out_names, zero_outs, mesh)
    return _cached[key]


def _prepare_in_arrays(x, W_lin, b_lin, wm, wf):
    """Host prep: per-core inputs concatenated over the core axis (axis 0)."""
    bf16 = ml_dtypes.bfloat16
    M = _band_matrix(wm, wf)

    # band blocks from the true M (out-of-range source columns are zero)
    mt_host = np.zeros((PT, TB, 2, PT), np.float32)
    for j in range(TB):
        rows = slice(j * PT, (j + 1) * PT)
        for side, base in ((0, j * PT - SHIFT), (1, j * PT + SHIFT)):
            lo = max(0, base)
            hi = min(L, base + PT)
            if hi > lo:
                # mt[ki, j, side, m] = M[j*128+m, base+ki]
                mt_host[lo - base:hi - base, j, side, :] = M[rows, lo:hi].T

    per_core = {
        "wT": np.ascontiguousarray(W_lin.T).astype(bf16),
        "mtB": mt_host.astype(bf16),
        "bvB": np.ascontiguousarray(
            np.broadcast_to(b_lin.reshape(1, H), (PT, H))).astype(bf16),
    }

    # shifted x tiles: xt[i][d, p] = x[b, t(i,p), d]
    tmap = np.empty((TB, PT), np.int64)
    for i in range(TB):
        for p in range(PT):
            tmap[i, p] = _t_of(i, p)
    arrays = {}
    xb = x.reshape(B, L, D)[:, tmap, :]                  # [B, TB, PT, D]
    xt = np.ascontiguousarray(xb.transpose(0, 1, 3, 2)).astype(bf16)
    arrays["xtT"] = xt.reshape(B * TB, D, PT)
    for name, arr in per_core.items():
        arrays[name] = np.concatenate([arr] * NCORES, axis=0)
    return arrays


def _run(arrays):
    fn, in_names, out_names, zero_outs, _ = _get_runner()
    global_zero = [np.concatenate([z] * NCORES, axis=0) for z in zero_outs]
    args = [arrays[n] for n in in_names] + global_zero
    outs = fn(*args)
    return {n: np.asarray(o) for n, o in zip(out_names, outs)}


def kernel(x, W_lin, b_lin, mem_w, la_w, gamma, beta):
    x = np.asarray(x, np.float32)
    W_lin = np.asarray(W_lin, np.float32)
    b_lin = np.asarray(b_lin, np.float32)
    wm = np.asarray(mem_w, np.float32).sum(axis=-1, dtype=np.float32)
    wf = np.asarray(la_w, np.float32).sum(axis=-1, dtype=np.float32)
    gamma = np.asarray(gamma, np.float32)
    beta = np.asarray(beta, np.float32)

    arrays = _prepare_in_arrays(x, W_lin, b_lin, wm, wf)
    outs = _run(arrays)
    out = outs["out"].reshape(NCORES, L, H)

    # gamma/beta affine (trivial for the spec's ones/zeros fills; exact in general)
    if not np.all(gamma == 1.0):
        out = out * gamma[None, None, :]
    if not np.all(beta == 0.0):
        out = out + beta[None, None, :]
    return np.ascontiguousarray(out.astype(np.float32))


# revision 19
# speedup vs baseline: 3.5149x; 1.1134x over previous
"""DFSMN layer Trainium2 kernel (8-core SPMD, batch-parallel).

Math: per batch b,
  h = x @ W^T + b_lin                      [L, H]
  out_pre[t] = h[t] + mem[t] + fut[t]  ==  (M @ h)[t]
    with M [L, L] banded: identity + past taps (50) + future taps (5),
    taps are scalars per lag: wm = mem_w.sum(-1), wf = la_w.sum(-1).
  out = LayerNorm_H(out_pre) * gamma + beta

On device (per core = one batch), all bf16 matmuls, fp32 PSUM:
  g tiles  = x @ W^T + b  produced on 64-SHIFTED time boundaries
             G_i = [i*128-64, i*128+64); the two half-empty edge windows are
             merged into one physical tile (partitions 0..63 = t in
             [1984,2048), partitions 64..127 = t in [0,64)).  Bias is folded
             by a DVE broadcast-add during PSUM evacuation (M@(g+1 b^T) =
             M@g + s b^T, which is exactly the reference bias path).
  band     = each aligned output tile j needs source window
             [j*128-50, j*128+133) which fits in G_j u G_{j+1}: only TWO
             128-contract band matmuls per (tile, H-chunk) instead of three.
             Band blocks are slices of the true M, so structural zeros mask
             the "wrong" halves of the merged edge tile.
  out      = (pre - mean) * rsqrt(var + eps) via bn_stats/bn_aggr; the final
             scale/shift runs on GpSimd (Pool) to keep DVE off the critical
             path.
"""
import numpy as np
import ml_dtypes

MEM, LA, EPS = 50, 5, 1e-5
B, L, D, H = 8, 2048, 1024, 2048
NCORES = 8
PT = 128              # time tile (partition dim)
TB = L // PT          # 16 time tiles
DC = D // PT          # 8 contract chunks
HN = 512              # matmul moving free dim
HC = H // HN          # 4 H chunks
SHIFT = 64            # g-tile shift

_cached = {}
last_exec_time_ns = None


def _band_matrix(wm, wf):
    """M [L, L] fp32: out_pre = M @ h."""
    M = np.zeros((L, L), np.float32)
    idx = np.arange(L)
    M[idx, idx] = 1.0
    for t in range(L):
        if t < MEM:
            M[t, :t] += wm[:t]
        else:
            M[t, t - MEM:t] += wm
        hi = min(t + LA, L - 1)
        if hi >= t + 1:
            M[t, t + 1:hi + 1] += wf[:hi - t]
    return M


def _t_of(i, p):
    """Global time index held at free-col/partition p of shifted g tile i."""
    if i == 0:
        return 1984 + p if p < SHIFT else p - SHIFT
    return i * PT - SHIFT + p


def _build_nc(reps=1, loop_k=None):
    from concourse import bacc
    import concourse.mybir as mybir
    import concourse.tile as tile

    dt = mybir.dt.bfloat16
    f32 = mybir.dt.float32

    nc = bacc.Bacc(None, target_bir_lowering=False)
    # x shipped transposed on shifted tile boundaries: [TB, D, PT]; tile i's
    # free col p holds x[t(i,p), :] (see _t_of).
    xtT = nc.declare_dram_parameter("xtT", [TB, D, PT], dt, isOutput=False)
    wT = nc.declare_dram_parameter("wT", [D, H], dt, isOutput=False)
    # band blocks: mtB[ki, j, 0, m] = M[j*128+m, j*128-64+ki]   (L source)
    #              mtB[ki, j, 1, m] = M[j*128+m, j*128+64+ki]   (R source)
    mtB = nc.declare_dram_parameter("mtB", [PT, TB, 2, PT], dt, isOutput=False)
    bvB = nc.declare_dram_parameter("bvB", [PT, H], dt, isOutput=False)
    # bf16 output (upcast on host): halves output HBM traffic; LN output is
    # ~unit-scale so bf16 rounding adds ~2e-3 rel err against a 2e-2 budget.
    out = nc.declare_dram_parameter("out", [L, H], dt, isOutput=True)

    with tile.TileContext(nc) as tc:
        with tc.tile_pool(name="const", bufs=1) as const, \
             tc.tile_pool(name="gm", bufs=1) as gmp, \
             tc.tile_pool(name="gpool", bufs=5) as gpool, \
             tc.tile_pool(name="opool", bufs=2) as opool, \
             tc.tile_pool(name="ln", bufs=2) as ln, \
             tc.tile_pool(name="psg", bufs=4, space="PSUM") as psg, \
             tc.tile_pool(name="psp", bufs=4, space="PSUM") as psp:

            # --- input loads: interleave W chunks and x tiles across the two
            # HWDGE queues (sync/scalar) so the first main chain's operands
            # (wt0..7 + xt0) land as early as possible; mt/bvb ride the
            # gpsimd SWDGE queue concurrently.
            wt_tiles = [const.tile([PT, H], dt, tag=f"wt{dc}", name=f"wt{dc}")
                        for dc in range(DC)]
            xt_tiles = [const.tile([PT, DC, PT], dt, tag=f"xt{i}", name=f"xt{i}")
                        for i in range(TB)]
            # xt0 first on scalar, then W alternating both queues, then the
            # remaining x tiles: chain0's deps (wt0..7+xt0) land as early as
            # possible; mt/bvb (needed only by band(0)/g-evac) ride the
            # gpsimd SWDGE queue and are emitted last so they don't delay
            # the critical W/x transfers.
            nc.scalar.dma_start(
                out=xt_tiles[0],
                in_=xtT[0].rearrange("(dc p) t -> p dc t", p=PT))
            for dc in range(DC):
                eng = nc.sync if dc % 2 == 0 else nc.scalar
                eng.dma_start(out=wt_tiles[dc],
                              in_=wT[dc * PT:(dc + 1) * PT, :])
            for i in range(1, TB):
                eng = nc.sync if i % 2 == 0 else nc.scalar
                eng.dma_start(out=xt_tiles[i],
                              in_=xtT[i].rearrange("(dc p) t -> p dc t", p=PT))
            mt_t = const.tile([PT, TB, 2, PT], dt, tag="mt")
            bvb_t = const.tile([PT, H], dt, tag="bvb")
            nc.gpsimd.dma_start(out=bvb_t, in_=bvB[:, :])
            nc.gpsimd.dma_start(out=mt_t, in_=mtB[:, :, :, :])
            eps_t = const.tile([PT, 1], f32, tag="eps")
            nc.vector.memset(eps_t, EPS)

            args = (nc, mybir, xt_tiles, wt_tiles, mt_t, bvb_t, eps_t,
                    gmp, gpool, opool, ln, psg, psp, out)
            if loop_k is not None:
                with tc.For_i(0, loop_k, 1):
                    _emit_body(*args)
            else:
                for _rep in range(reps):
                    _emit_body(*args)
    nc.finalize()
    return nc


def _emit_body(nc, mybir, xt_tiles, wt_tiles, mt_t, bvb_t, eps_t,
               gmp, gpool, opool, ln, psg, psp, out):
    dt = mybir.dt.bfloat16
    f32 = mybir.dt.float32
    sub = mybir.AluOpType.subtract
    mult = mybir.AluOpType.mult
    add = mybir.AluOpType.add

    g_tiles = [None] * TB

    def emit_band(j):
        gL = g_tiles[j]
        gR = g_tiles[(j + 1) % TB]
        stats = ln.tile([PT, HC, 6], f32, tag="stats")
        pres = [psp.tile([PT, HN], f32, tag="pre", name=f"pre{j}_{hc}")
                for hc in range(HC)]
        # All L matmuls then all R matmuls: the stationary band block is
        # loaded once per group of four instead of per matmul.
        for hc in range(HC):
            nc.tensor.matmul(pres[hc], mt_t[:, j, 0, :], gL[hc],
                             start=True, stop=False)
        for hc in range(HC):
            nc.tensor.matmul(pres[hc], mt_t[:, j, 1, :], gR[hc],
                             start=False, stop=True)
        for hc in range(HC):
            # LN runs straight out of PSUM: bn_stats (DVE) now, the final
            # scale/shift (ScalarE) later — no evacuation copy at all.
            nc.vector.bn_stats(out=stats[:, hc, :], in_=pres[hc])
        mv = ln.tile([PT, 2], f32, tag="mv")
        nc.vector.bn_aggr(out=mv, in_=stats)
        rstd = ln.tile([PT, 1], f32, tag="rstd")
        nc.scalar.activation(
            out=rstd, in_=mv[:, 1:2],
            func=mybir.ActivationFunctionType.Sqrt,
            bias=eps_t, scale=1.0)
        nc.vector.reciprocal(out=rstd, in_=rstd)
        # nmr = -mean * rstd so the final scale/shift can run on ScalarE as
        # one Identity activation: o = pre * rstd + nmr.
        nmr = ln.tile([PT, 1], f32, tag="nmr")
        nc.vector.tensor_scalar(
            out=nmr, in0=mv[:, 0:1], scalar1=rstd, scalar2=-1.0,
            op0=mult, op1=mult)
        for hc in range(HC):
            # Output DMA per H-chunk so the last tile's store overlaps its LN.
            o = opool.tile([PT, HN], dt, tag=f"o{hc}", name=f"o{hc}")
            nc.scalar.activation(
                out=o, in_=pres[hc],
                func=mybir.ActivationFunctionType.Identity,
                bias=nmr, scale=rstd)
            eng = nc.sync if (j * HC + hc) % 2 == 0 else nc.scalar
            eng.dma_start(
                out=out[j * PT:(j + 1) * PT, hc * HN:(hc + 1) * HN], in_=o)

    for i in range(TB):
        # main chain i -> shifted g tile i.  dc-outer order: the stationary
        # x chunk is reused across the 4 H-chunk PSUM chains (fewer
        # LDWEIGHTS), chains run in 4 PSUM banks concurrently.
        pgs = [psg.tile([PT, HN], f32, tag="pg", name=f"pg{i}_{hc}")
               for hc in range(HC)]
        for dc in range(DC):
            for hc in range(HC):
                nc.tensor.matmul(
                    pgs[hc],
                    xt_tiles[i][:, dc, :],
                    wt_tiles[dc][:, hc * HN:(hc + 1) * HN],
                    start=(dc == 0), stop=(dc == DC - 1))
        gch = []
        for hc in range(HC):
            pool = gmp if i == 0 else gpool
            g = pool.tile([PT, HN], dt, tag=(f"gm{hc}" if i == 0 else f"g{hc}"))
            # evacuate PSUM on DVE with the bias broadcast-add folded in
            nc.vector.tensor_tensor(
                out=g, in0=pgs[hc], in1=bvb_t[:, hc * HN:(hc + 1) * HN],
                op=add)
            gch.append(g)
        g_tiles[i] = gch
        # band(j) waits one extra chain (emitted after chain j+2) so the DVE
        # evacuation of g(j+1) has a full chain of PE work to hide under.
        # band(TB-3) is pulled forward (delay 1) so that at most two bands
        # remain after the last chain and their PSUM banks are already free.
        if i >= 2:
            emit_band(i - 2)
        if i == TB - 2:
            emit_band(TB - 3)
    emit_band(TB - 2)
    emit_band(TB - 1)


def _get_runner(reps=1):
    """Compile once; return (run_fn, in_names, out_names, zero_outs, mesh)."""
    key = ("runner", reps)
    if key in _cached:
        return _cached[key]

    import jax
    from jax.experimental.shard_map import shard_map
    from jax.sharding import Mesh, PartitionSpec
    import concourse.mybir as mybir
    from concourse import bass2jax

    if isinstance(reps, tuple):  # ("loop", K): hardware For_i timing variant
        nc = _build_nc(loop_k=reps[1])
    else:
        nc = _build_nc(reps)
    bass2jax.install_neuronx_cc_hook()

    partition_name = nc.partition_id_tensor.name if nc.partition_id_tensor else None
    in_names, out_names, out_avals, zero_outs = [], [], [], []
    for alloc in nc.m.functions[0].allocations:
        if not isinstance(alloc, mybir.MemoryLocationSet):
            continue
        name = alloc.memorylocations[0].name
        if alloc.kind == "ExternalInput":
            if name != partition_name:
                in_names.append(name)
        elif alloc.kind == "ExternalOutput":
            out_names.append(name)
            shape = tuple(alloc.tensor_shape)
            dtype = mybir.dt.np(alloc.dtype)
            out_avals.append(jax.core.ShapedArray(shape, dtype))
            zero_outs.append(np.zeros(shape, dtype))
    n_params = len(in_names)
    all_names = in_names + out_names
    if partition_name is not None:
        all_names.append(partition_name)

    def _body(*args):
        operands = list(args)
        if partition_name is not None:
            operands.append(bass2jax.partition_id_tensor())
        outs = bass2jax._bass_exec_p.bind(
            *operands,
            out_avals=tuple(out_avals),
            in_names=tuple(all_names),
            out_names=tuple(out_names),
            lowering_input_output_aliases=(),
            sim_require_finite=True,
            sim_require_nnan=True,
            nc=nc,
        )
        return tuple(outs)

    devices = jax.devices()[:NCORES]
    assert len(devices) == NCORES, f"need {NCORES} devices, have {len(jax.devices())}"
    mesh = Mesh(np.asarray(devices), ("core",))
    n_outs = len(out_names)
    fn = jax.jit(shard_map(
        _body, mesh=mesh,
        in_specs=(PartitionSpec("core"),) * (n_params + n_outs),
        out_specs=(PartitionSpec("core"),) * n_outs,
        check_rep=False))

    _cached[key] = (fn, in_names, out_names, zero_outs, mesh)
    return _cached[key]


def _prepare_in_arrays(x, W_lin, b_lin, wm, wf):
    """Host prep: per-core inputs concatenated over the core axis (axis 0)."""
    bf16 = ml_dtypes.bfloat16
    M = _band_matrix(wm, wf)

    # band blocks from the true M (out-of-range source columns are zero)
    mt_host = np.zeros((PT, TB, 2, PT), np.float32)
    for j in range(TB):
        rows = slice(j * PT, (j + 1) * PT)
        for side, base in ((0, j * PT - SHIFT), (1, j * PT + SHIFT)):
            lo = max(0, base)
            hi = min(L, base + PT)
            if hi > lo:
                # mt[ki, j, side, m] = M[j*128+m, base+ki]
                mt_host[lo - base:hi - base, j, side, :] = M[rows, lo:hi].T

    per_core = {
        "wT": np.ascontiguousarray(W_lin.T).astype(bf16),
        "mtB": mt_host.astype(bf16),
        "bvB": np.ascontiguousarray(
            np.broadcast_to(b_lin.reshape(1, H), (PT, H))).astype(bf16),
    }

    # shifted x tiles: xt[i][d, p] = x[b, t(i,p), d]
    tmap = np.empty((TB, PT), np.int64)
    for i in range(TB):
        for p in range(PT):
            tmap[i, p] = _t_of(i, p)
    arrays = {}
    xb = x.reshape(B, L, D)[:, tmap, :]                  # [B, TB, PT, D]
    xt = np.ascontiguousarray(xb.transpose(0, 1, 3, 2)).astype(bf16)
    arrays["xtT"] = xt.reshape(B * TB, D, PT)
    for name, arr in per_core.items():
        arrays[name] = np.concatenate([arr] * NCORES, axis=0)
    return arrays


def _run(arrays):
    fn, in_names, out_names, zero_outs, _ = _get_runner()
    global_zero = [np.concatenate([z] * NCORES, axis=0) for z in zero_outs]
    args = [arrays[n] for n in in_names] + global_zero
    outs = fn(*args)
    return {n: np.asarray(o) for n, o in zip(out_names, outs)}


def kernel(x, W_lin, b_lin, mem_w, la_w, gamma, beta):
    x = np.asarray(x, np.float32)
    W_lin = np.asarray(W_lin, np.float32)
    b_lin = np.asarray(b_lin, np.float32)
    wm = np.asarray(mem_w, np.float32).sum(axis=-1, dtype=np.float32)
    wf = np.asarray(la_w, np.float32).sum(axis=-1, dtype=np.float32)
    gamma = np.asarray(gamma, np.float32)
    beta = np.asarray(beta, np.float32)

    arrays = _prepare_in_arrays(x, W_lin, b_lin, wm, wf)
    outs = _run(arrays)
    out = outs["out"].reshape(NCORES, L, H)

    # gamma/beta affine (trivial for the spec's ones/zeros fills; exact in general)
    if not np.all(gamma == 1.0):
        out = out * gamma[None, None, :]
    if not np.all(beta == 0.0):
        out = out + beta[None, None, :]
    return np.ascontiguousarray(out.astype(np.float32))
